# revision 10
# baseline (speedup 1.0000x reference)
"""CVQNN classifier kernel for 8 Trainium2 NeuronCores.

Math: the whole quantum circuit collapses to a batch-independent affine map
(S, d) on 128-dim phase space.  Per batch row the heavy work is
    msel' = x @ W2 + d20          (W2 = S[rows, :64].T, shape (64, 20))
    out_k = log1p(msel'_x[k]^2 + msel'_p[k]^2 + cov_k/4 - 0.5)
i.e. a (B,64) @ (64,20) matmul + elementwise tail -> (B,10).  Memory bound:
minimize HBM bytes (fp16 in, fp16 out; gate is 2e-2, fp16 end-to-end is
~5e-4) and keep the 16 DMA engines saturated end-to-end.

Device layout (per core, R = 125184 rows = 489 pair-blocks of 256):
  - host packs xstack (128, R/2) fp16, "2-pack": column c = (pair b,
    lane l), partitions 0..63 = features of row 256b+l, partitions
    64..127 = features of row 256b+128+l.  Full 128 partitions keeps
    DMA descriptors on all 16 engines (a 65-partition layout was
    measured to use only 13 and run ~25% slower per descriptor) and
    halves the LDWEIGHTS count (one stationary load per 256 rows).
  - per super-block of `jblk` pair-blocks: 1 DMA [128, 128*jblk] fp16
    (12 KB/partition descriptors at jblk=48 — the shape measured at
    ~19.7 B/ns/engine), one matmul per pair-block: stationary =
    xstack_b [128, 128], moving = wcat [128, 40] = [[W2,0],[0,W2]],
    psum cols = [Ax Ap Bx Bp] x 10.  12 pair-blocks per 512-col psum
    bank (480 cols used), up to 4 banks/super-block, double-buffered.
  - tail: t2 = psum + d (DVE, fp16 out), sq = t2^2 (ACT), s = sq_x +
    sq_p (DVE fp16), v = s + covc (DVE fp16), o = ln(1+v) (ACT, fp16).
    relu is dropped: nmean >= 0 exactly (mean photon number), and v is
    a sum of nonnegative fp16 terms so ln(1+v) is always finite.
  - out DMA [128, 20*jblk] fp16 on the scalar HWDGE queue: output never
    queues behind input loads and there is no gpsimd SWDGE drain.
  - widths taper [12, 24, 48*8, 36, 24, 9] pair-blocks: small first
    block starts compute early; small last blocks shrink the post-DMA
    pipeline drain.
"""

import numpy as np

import concourse.bacc as bacc
import concourse.mybir as mybir
import concourse.tile as tile
from concourse.bass_utils import run_bass_kernel_spmd

N = 64          # wires
OUT = 10        # measured wires / classes
NCORES = 8
PPB = 12                       # pair-blocks per psum bank (12*40 = 480 cols)
WIDTHS = [12, 24] + [48] * 8 + [36, 18, 9, 6]  # pair-blocks per super-block
NP2 = sum(WIDTHS)              # 489 pair-blocks
NJ = 2 * NP2                   # 978 j-blocks of 128 rows
R = 128 * NJ                   # per-core rows = 125184
B_PAD = R * NCORES             # 1001472
F32 = mybir.dt.float32
F16 = mybir.dt.float16
NPF16 = np.float16


# ---------------------------------------------------------------- host math
def _bs_pass(n, start, int_params):
    i = np.arange(start, n - 1, 2)
    j = i + 1
    theta = int_params[3 * i]
    phi = int_params[3 * i + 1]
    ct, st = np.cos(theta), np.sin(theta)
    cp, sp = np.cos(phi), np.sin(phi)
    S = np.eye(2 * n)
    S[i, i] = ct
    S[i, j] = -cp * st
    S[i, n + j] = -sp * st
    S[j, i] = cp * st
    S[j, j] = ct
    S[j, n + i] = -sp * st
    S[n + i, j] = sp * st
    S[n + i, n + i] = ct
    S[n + i, n + j] = -cp * st
    S[n + j, i] = sp * st
    S[n + j, n + i] = cp * st
    S[n + j, n + j] = ct
    return S


def _layer_symplectic(n, int1, squeezes, int2):
    M = _bs_pass(n, 0, int1)
    M = _bs_pass(n, 1, int1) @ M
    c = np.concatenate([np.cos(int1[2::3]), np.ones(1)])
    s = np.concatenate([np.sin(int1[2::3]), np.zeros(1)])
    Rm = np.block([[np.diag(c), np.diag(-s)], [np.diag(s), np.diag(c)]])
    Sq = np.diag(np.concatenate([np.exp(-squeezes), np.exp(squeezes)]))
    M = Sq @ (Rm @ M)
    M = _bs_pass(n, 0, int2) @ M
    M = _bs_pass(n, 1, int2) @ M
    return M


def _affine_map(layers):
    n = N
    S = np.eye(2 * n)
    d = np.zeros(2 * n)
    for int1, sq, int2, disp in layers:
        M = _layer_symplectic(n, int1, sq, int2)
        S = M @ S
        d = M @ d
        d[:n] += 2.0 * disp
    return S, d


def _device_constants(layers):
    S, d = _affine_map(layers)
    w = np.arange(OUT)
    rows = np.concatenate([w, N + w])
    cov = S @ S.T
    cov_term = cov[w, w] + cov[N + w, N + w]            # (10,)
    W2 = S[rows, :N].T                                  # (64, 20), msel' scale
    d20 = d[rows] / 2.0                                 # (20,)
    covc = (cov_term / 4.0 - 0.5).astype(np.float32)    # (10,)

    wcat = np.zeros((128, 40), NPF16)                   # [[W2, 0], [0, W2]]
    wcat[0:64, 0:20] = W2.astype(NPF16)
    wcat[64:128, 20:40] = W2.astype(NPF16)

    dconst = np.ascontiguousarray(np.broadcast_to(
        np.tile(d20.astype(np.float32), 2 * 4 * PPB),
        (128, 40 * 4 * PPB))).astype(np.float32)
    cconst = np.ascontiguousarray(np.broadcast_to(
        np.tile(covc, 2 * 4 * PPB), (128, 20 * 4 * PPB))).astype(NPF16)
    return wcat, dconst, cconst


# ---------------------------------------------------------------- bass build
def build_nc(widths=None):
    widths = widths or WIDTHS
    np2 = sum(widths)
    cc = 128 * np2                              # xstack cols
    nc = bacc.Bacc("TRN2", target_bir_lowering=False)
    WC = 40 * 4 * PPB                           # psum cols per full SB (1920)
    OC = 20 * 4 * PPB                           # out cols per full SB (960)
    xs = nc.dram_tensor("xs", (128, cc), F16, kind="ExternalInput")
    wst = nc.dram_tensor("wcat", (128, 40), F16, kind="ExternalInput")
    dcon = nc.dram_tensor("dconst", (128, WC), F32, kind="ExternalInput")
    ccon = nc.dram_tensor("covconst", (128, OC), F16, kind="ExternalInput")
    out = nc.dram_tensor("out", (128, 2 * np2 * OUT), F16,
                         kind="ExternalOutput")

    Square = mybir.ActivationFunctionType.Square
    Ln = mybir.ActivationFunctionType.Ln

    with tile.TileContext(nc) as tc:
        with (
            tc.tile_pool(name="const", bufs=1) as cpool,
            tc.tile_pool(name="xin", bufs=8) as xpool,
            tc.tile_pool(name="mid", bufs=3) as mpool,
            tc.tile_pool(name="ob", bufs=4) as opool,
            tc.tile_pool(name="ps", bufs=4, space="PSUM") as pspool,
        ):
            # w_t gates the first matmul: load it first on the sync queue;
            # d/c consts go on the scalar queue so they don't delay x.
            w_t = cpool.tile([128, 40], F16)
            nc.sync.dma_start(w_t[:], wst[:])
            d_t = cpool.tile([128, WC], F32)
            nc.scalar.dma_start(d_t[:], dcon[:])
            c_t = cpool.tile([128, OC], F16)
            nc.scalar.dma_start(c_t[:], ccon[:])

            def emit_sb(col_base, jblk, in_chunks):
                wc, oc = 40 * jblk, 20 * jblk
                nbank = (jblk + PPB - 1) // PPB
                w = 128 * jblk
                tin = xpool.tile([128, w], F16, tag="tin")
                q = w // in_chunks
                for c4 in range(in_chunks):
                    nc.sync.dma_start(
                        tin[:, c4 * q:(c4 + 1) * q],
                        xs[:, col_base + c4 * q:col_base + (c4 + 1) * q])

                # psum: 12 pair-blocks use the first 480 cols of each
                # 512-col bank (no bank crossing).  Chunks of <= 24 pairs
                # (2 banks) with bufs=4 keep two super-blocks of matmuls
                # in flight and recycle banks as soon as the per-chunk
                # d-add has drained them.
                t2 = mpool.tile([128, wc], F16, tag="t2")
                base = 0
                left = jblk
                while left:
                    cp = min(2 * PPB, left)
                    nb = (cp + PPB - 1) // PPB
                    ps = pspool.tile([128, nb, 512], F32, tag="ps")
                    for j in range(cp):
                        jj = base + j
                        nc.tensor.matmul(
                            ps[:, j // PPB, 40 * (j % PPB):40 * (j % PPB) + 40],
                            tin[:, 128 * jj:128 * jj + 128], w_t[:],
                            start=True, stop=True,
                        )
                    fullb = cp // PPB
                    remj = cp - fullb * PPB
                    off = 40 * base
                    if fullb:
                        pv = ps[:, 0:fullb, 0:40 * PPB]
                        tv = t2[:, off:off + 40 * fullb * PPB].rearrange(
                            "p (t q) -> p t q", t=fullb)
                        dv = d_t[:, 0:40 * fullb * PPB].rearrange(
                            "p (t q) -> p t q", t=fullb)
                        nc.vector.tensor_add(tv, pv, dv)
                    if remj:
                        pv = ps[:, fullb, 0:40 * remj]
                        tv = t2[:, off + 40 * fullb * PPB:off + 40 * cp]
                        nc.vector.tensor_add(tv, pv, d_t[:, 0:40 * remj])
                    base += cp
                    left -= cp

                sq = mpool.tile([128, wc], F16, tag="sq")
                nc.scalar.activation(sq[:], t2[:], Square)
                sqv = sq[:].rearrange("p (g r k) -> p g r k", r=2, k=OUT)
                s = mpool.tile([128, oc], F16, tag="s")
                sv = s[:].rearrange("p (g k) -> p g k", k=OUT)
                # gpsimd is ~2.6x slower per element than DVE but otherwise
                # idle: give it the pair-add on full blocks to balance the
                # engines; keep the taper blocks on DVE for a short drain.
                seng = nc.gpsimd if jblk >= 4 * PPB else nc.vector
                seng.tensor_add(sv, sqv[:, :, 0, :], sqv[:, :, 1, :])
                v = mpool.tile([128, oc], F16, tag="v")
                nc.vector.tensor_add(v[:], s[:], c_t[:, 0:oc])
                o = opool.tile([128, oc], F16, tag="o")
                nc.scalar.activation(o[:], v[:], Ln, bias=1.0)

                ob = (col_base // 128) * 20
                nc.sync.dma_start(out[:, ob:ob + oc], o[:])

            # first tile's DMA in halves so compute starts sooner
            col = 0
            for i, wdt in enumerate(widths):
                emit_sb(col, wdt, 2 if i == 0 else 1)
                col += 128 * wdt
    nc.compile()
    return nc


# ---------------------------------------------------------------- host glue
def _make_in_maps(x_batch, wcat, dconst, cconst):
    B = x_batch.shape[0]
    xpad = np.zeros((B_PAD, N), NPF16)
    xpad[:B] = x_batch
    in_maps = []
    for c in range(NCORES):
        xc = xpad[c * R:(c + 1) * R]
        # xstk[64*m + f, 128*b + l] = xc[256*b + 128*m + l, f]
        xstk = np.ascontiguousarray(
            xc.reshape(R // 256, 2, 128, N).transpose(1, 3, 0, 2)
            .reshape(128, R // 2))
        in_maps.append({"xs": xstk, "wcat": wcat,
                        "dconst": dconst, "covconst": cconst})
    return in_maps


def _decode_out(results, B):
    full = np.empty((B_PAD, OUT), np.float32)
    for c in range(NCORES):
        O = results[c]["out"].astype(np.float32).reshape(128, NJ, OUT)
        rows = O.transpose(1, 0, 2).reshape(R, OUT)
        full[c * R:(c + 1) * R] = rows
    return full[:B]


_NC_CACHE = {}


def kernel(x_batch, int1_0, squeezes_0, int2_0, disp_0,
           int1_1, squeezes_1, int2_1, disp_1, _trace=False):
    layers = [
        (np.asarray(int1_0, np.float64), np.asarray(squeezes_0, np.float64),
         np.asarray(int2_0, np.float64), np.asarray(disp_0, np.float64)),
        (np.asarray(int1_1, np.float64), np.asarray(squeezes_1, np.float64),
         np.asarray(int2_1, np.float64), np.asarray(disp_1, np.float64)),
    ]
    wcat, dconst, cconst = _device_constants(layers)
    in_maps = _make_in_maps(np.asarray(x_batch, np.float32), wcat, dconst,
                            cconst)

    if "nc" not in _NC_CACHE:
        _NC_CACHE["nc"] = build_nc()
    nc = _NC_CACHE["nc"]

    res = run_bass_kernel_spmd(
        nc, in_maps, core_ids=list(range(NCORES)), trace=_trace
    )
    out = _decode_out(res.results, x_batch.shape[0])
    if _trace:
        return out, res
    return out


# revision 11
# speedup vs baseline: 1.0585x; 1.0585x over previous
"""CVQNN classifier kernel for 8 Trainium2 NeuronCores.

Math: the whole quantum circuit collapses to a batch-independent affine map
(S, d) on 128-dim phase space.  Per batch row the heavy work is
    msel' = x @ W2 + d20          (W2 = S[rows, :64].T, shape (64, 20))
    out_k = log1p(msel'_x[k]^2 + msel'_p[k]^2 + cov_k/4 - 0.5)
i.e. a (B,64) @ (64,20) matmul + elementwise tail -> (B,10).  Memory bound:
minimize HBM bytes (fp16 in, fp16 out; gate is 2e-2, fp16 end-to-end is
~5e-4) and keep the 16 DMA engines saturated end-to-end.

Device layout (per core, R = 125184 rows = 489 pair-blocks of 256):
  - host packs xstack (128, R/2) fp16, "2-pack": column c = (pair b,
    lane l), partitions 0..63 = features of row 256b+l, partitions
    64..127 = features of row 256b+128+l.  Full 128 partitions keeps
    DMA descriptors on all 16 engines (a 65-partition layout was
    measured to use only 13 and run ~25% slower per descriptor) and
    halves the LDWEIGHTS count (one stationary load per 256 rows).
  - per super-block of `jblk` pair-blocks: 1 DMA [128, 128*jblk] fp16
    (12 KB/partition descriptors at jblk=48 — the shape measured at
    ~19.7 B/ns/engine), one matmul per pair-block: stationary =
    xstack_b [128, 128], moving = wcat [128, 40] = [[W2,0],[0,W2]],
    psum cols = [Ax Ap Bx Bp] x 10.  12 pair-blocks per 512-col psum
    bank (480 cols used), up to 4 banks/super-block, double-buffered.
  - tail: t2 = psum + d (DVE, fp16 out), sq = t2^2 (ACT), s = sq_x +
    sq_p (DVE fp16), v = s + covc (DVE fp16), o = ln(1+v) (ACT, fp16).
    relu is dropped: nmean >= 0 exactly (mean photon number), and v is
    a sum of nonnegative fp16 terms so ln(1+v) is always finite.
  - out DMA [128, 20*jblk] fp16 on the scalar HWDGE queue: output never
    queues behind input loads and there is no gpsimd SWDGE drain.
  - widths taper [12, 24, 48*8, 36, 24, 9] pair-blocks: small first
    block starts compute early; small last blocks shrink the post-DMA
    pipeline drain.
"""

import numpy as np

import concourse.bacc as bacc
import concourse.mybir as mybir
import concourse.tile as tile
from concourse.bass_utils import run_bass_kernel_spmd

N = 64          # wires
OUT = 10        # measured wires / classes
NCORES = 8
PPB = 12                       # pair-blocks per psum bank (12*40 = 480 cols)
WIDTHS = [12, 24] + [48] * 8 + [36, 18, 9, 6]  # pair-blocks per super-block
NP2 = sum(WIDTHS)              # 489 pair-blocks
NJ = 2 * NP2                   # 978 j-blocks of 128 rows
R = 128 * NJ                   # per-core rows = 125184
B_PAD = R * NCORES             # 1001472
F32 = mybir.dt.float32
F16 = mybir.dt.float16
NPF16 = np.float16


# ---------------------------------------------------------------- host math
def _bs_pass(n, start, int_params):
    i = np.arange(start, n - 1, 2)
    j = i + 1
    theta = int_params[3 * i]
    phi = int_params[3 * i + 1]
    ct, st = np.cos(theta), np.sin(theta)
    cp, sp = np.cos(phi), np.sin(phi)
    S = np.eye(2 * n)
    S[i, i] = ct
    S[i, j] = -cp * st
    S[i, n + j] = -sp * st
    S[j, i] = cp * st
    S[j, j] = ct
    S[j, n + i] = -sp * st
    S[n + i, j] = sp * st
    S[n + i, n + i] = ct
    S[n + i, n + j] = -cp * st
    S[n + j, i] = sp * st
    S[n + j, n + i] = cp * st
    S[n + j, n + j] = ct
    return S


def _layer_symplectic(n, int1, squeezes, int2):
    M = _bs_pass(n, 0, int1)
    M = _bs_pass(n, 1, int1) @ M
    c = np.concatenate([np.cos(int1[2::3]), np.ones(1)])
    s = np.concatenate([np.sin(int1[2::3]), np.zeros(1)])
    Rm = np.block([[np.diag(c), np.diag(-s)], [np.diag(s), np.diag(c)]])
    Sq = np.diag(np.concatenate([np.exp(-squeezes), np.exp(squeezes)]))
    M = Sq @ (Rm @ M)
    M = _bs_pass(n, 0, int2) @ M
    M = _bs_pass(n, 1, int2) @ M
    return M


def _affine_map(layers):
    n = N
    S = np.eye(2 * n)
    d = np.zeros(2 * n)
    for int1, sq, int2, disp in layers:
        M = _layer_symplectic(n, int1, sq, int2)
        S = M @ S
        d = M @ d
        d[:n] += 2.0 * disp
    return S, d


def _device_constants(layers):
    S, d = _affine_map(layers)
    w = np.arange(OUT)
    rows = np.concatenate([w, N + w])
    cov = S @ S.T
    cov_term = cov[w, w] + cov[N + w, N + w]            # (10,)
    W2 = S[rows, :N].T                                  # (64, 20), msel' scale
    d20 = d[rows] / 2.0                                 # (20,)
    covc = (cov_term / 4.0 - 0.5).astype(np.float32)    # (10,)

    wcat = np.zeros((128, 40), NPF16)                   # [[W2, 0], [0, W2]]
    wcat[0:64, 0:20] = W2.astype(NPF16)
    wcat[64:128, 20:40] = W2.astype(NPF16)

    dconst = np.ascontiguousarray(np.broadcast_to(
        np.tile(d20.astype(np.float32), 2 * 4 * PPB),
        (128, 40 * 4 * PPB))).astype(np.float32)
    cconst = np.ascontiguousarray(np.broadcast_to(
        np.tile(covc, 2 * 4 * PPB), (128, 20 * 4 * PPB))).astype(NPF16)
    return wcat, dconst, cconst


# ---------------------------------------------------------------- bass build
def build_nc(widths=None):
    widths = widths or WIDTHS
    np2 = sum(widths)
    cc = 128 * np2                              # xstack cols
    nc = bacc.Bacc("TRN2", target_bir_lowering=False)
    WC = 40 * 4 * PPB                           # psum cols per full SB (1920)
    OC = 20 * 4 * PPB                           # out cols per full SB (960)
    xs = nc.dram_tensor("xs", (128, cc), F16, kind="ExternalInput")
    wst = nc.dram_tensor("wcat", (128, 40), F16, kind="ExternalInput")
    dcon = nc.dram_tensor("dconst", (128, WC), F32, kind="ExternalInput")
    ccon = nc.dram_tensor("covconst", (128, OC), F16, kind="ExternalInput")
    out = nc.dram_tensor("out", (128, 2 * np2 * OUT), F16,
                         kind="ExternalOutput")

    Square = mybir.ActivationFunctionType.Square
    Ln = mybir.ActivationFunctionType.Ln

    with tile.TileContext(nc) as tc:
        with (
            tc.tile_pool(name="const", bufs=1) as cpool,
            tc.tile_pool(name="xin", bufs=8) as xpool,
            tc.tile_pool(name="mid", bufs=3) as mpool,
            tc.tile_pool(name="ob", bufs=4) as opool,
            tc.tile_pool(name="ps", bufs=4, space="PSUM") as pspool,
        ):
            # w_t gates the first matmul: load it first on the sync queue;
            # d/c consts go on the scalar queue so they don't delay x.
            w_t = cpool.tile([128, 40], F16)
            nc.sync.dma_start(w_t[:], wst[:])
            d_t = cpool.tile([128, WC], F32)
            nc.scalar.dma_start(d_t[:], dcon[:])
            c_t = cpool.tile([128, OC], F16)
            nc.scalar.dma_start(c_t[:], ccon[:])

            def emit_tail(prev):
                # tail of the PREVIOUS super-block: by now its sq is long
                # done, so none of these ops ever stalls at the head of an
                # in-order engine queue (the queues run strictly in order,
                # so an op waiting on a slow cross-engine dependency blocks
                # everything emitted after it on the same queue).
                sq, col_base, jblk = prev
                oc = 20 * jblk
                sqv = sq[:].rearrange("p (g r k) -> p g r k", r=2, k=OUT)
                s = mpool.tile([128, oc], F16, tag="s")
                sv = s[:].rearrange("p (g k) -> p g k", k=OUT)
                nc.vector.tensor_add(sv, sqv[:, :, 0, :], sqv[:, :, 1, :])
                v = mpool.tile([128, oc], F16, tag="v")
                nc.vector.tensor_add(v[:], s[:], c_t[:, 0:oc])
                o = opool.tile([128, oc], F16, tag="o")
                nc.scalar.activation(o[:], v[:], Ln, bias=1.0)
                ob = (col_base // 128) * 20
                nc.scalar.dma_start(out[:, ob:ob + oc], o[:])

            def emit_sb(col_base, jblk, in_chunks, prev):
                wc = 40 * jblk
                w = 128 * jblk
                tin = xpool.tile([128, w], F16, tag="tin")
                q = w // in_chunks
                for c4 in range(in_chunks):
                    nc.sync.dma_start(
                        tin[:, c4 * q:(c4 + 1) * q],
                        xs[:, col_base + c4 * q:col_base + (c4 + 1) * q])

                if prev is not None:
                    emit_tail(prev)

                # psum: 12 pair-blocks use the first 480 cols of each
                # 512-col bank (no bank crossing).  Chunks of <= 24 pairs
                # (2 banks) with bufs=4 keep two super-blocks of matmuls
                # in flight and recycle banks as soon as the per-chunk
                # d-add has drained them.
                t2 = mpool.tile([128, wc], F16, tag="t2")
                base = 0
                left = jblk
                while left:
                    cp = min(2 * PPB, left)
                    nb = (cp + PPB - 1) // PPB
                    ps = pspool.tile([128, nb, 512], F32, tag="ps")
                    for j in range(cp):
                        jj = base + j
                        nc.tensor.matmul(
                            ps[:, j // PPB, 40 * (j % PPB):40 * (j % PPB) + 40],
                            tin[:, 128 * jj:128 * jj + 128], w_t[:],
                            start=True, stop=True,
                        )
                    fullb = cp // PPB
                    remj = cp - fullb * PPB
                    off = 40 * base
                    if fullb:
                        pv = ps[:, 0:fullb, 0:40 * PPB]
                        tv = t2[:, off:off + 40 * fullb * PPB].rearrange(
                            "p (t q) -> p t q", t=fullb)
                        dv = d_t[:, 0:40 * fullb * PPB].rearrange(
                            "p (t q) -> p t q", t=fullb)
                        nc.vector.tensor_add(tv, pv, dv)
                    if remj:
                        pv = ps[:, fullb, 0:40 * remj]
                        tv = t2[:, off + 40 * fullb * PPB:off + 40 * cp]
                        nc.vector.tensor_add(tv, pv, d_t[:, 0:40 * remj])
                    base += cp
                    left -= cp

                sq = mpool.tile([128, wc], F16, tag="sq")
                nc.scalar.activation(sq[:], t2[:], Square)
                return (sq, col_base, jblk)

            # first tile's DMA in halves so compute starts sooner
            col = 0
            prev = None
            for i, wdt in enumerate(widths):
                prev = emit_sb(col, wdt, 2 if i == 0 else 1, prev)
                col += 128 * wdt
            emit_tail(prev)
    nc.compile()
    return nc


# ---------------------------------------------------------------- host glue
def _make_in_maps(x_batch, wcat, dconst, cconst):
    B = x_batch.shape[0]
    xpad = np.zeros((B_PAD, N), NPF16)
    xpad[:B] = x_batch
    in_maps = []
    for c in range(NCORES):
        xc = xpad[c * R:(c + 1) * R]
        # xstk[64*m + f, 128*b + l] = xc[256*b + 128*m + l, f]
        xstk = np.ascontiguousarray(
            xc.reshape(R // 256, 2, 128, N).transpose(1, 3, 0, 2)
            .reshape(128, R // 2))
        in_maps.append({"xs": xstk, "wcat": wcat,
                        "dconst": dconst, "covconst": cconst})
    return in_maps


def _decode_out(results, B):
    full = np.empty((B_PAD, OUT), np.float32)
    for c in range(NCORES):
        O = results[c]["out"].astype(np.float32).reshape(128, NJ, OUT)
        rows = O.transpose(1, 0, 2).reshape(R, OUT)
        full[c * R:(c + 1) * R] = rows
    return full[:B]


_NC_CACHE = {}


def kernel(x_batch, int1_0, squeezes_0, int2_0, disp_0,
           int1_1, squeezes_1, int2_1, disp_1, _trace=False):
    layers = [
        (np.asarray(int1_0, np.float64), np.asarray(squeezes_0, np.float64),
         np.asarray(int2_0, np.float64), np.asarray(disp_0, np.float64)),
        (np.asarray(int1_1, np.float64), np.asarray(squeezes_1, np.float64),
         np.asarray(int2_1, np.float64), np.asarray(disp_1, np.float64)),
    ]
    wcat, dconst, cconst = _device_constants(layers)
    in_maps = _make_in_maps(np.asarray(x_batch, np.float32), wcat, dconst,
                            cconst)

    if "nc" not in _NC_CACHE:
        _NC_CACHE["nc"] = build_nc()
    nc = _NC_CACHE["nc"]

    res = run_bass_kernel_spmd(
        nc, in_maps, core_ids=list(range(NCORES)), trace=_trace
    )
    out = _decode_out(res.results, x_batch.shape[0])
    if _trace:
        return out, res
    return out


# revision 12
# speedup vs baseline: 1.0654x; 1.0065x over previous
"""CVQNN classifier kernel for 8 Trainium2 NeuronCores.

Math: the whole quantum circuit collapses to a batch-independent affine map
(S, d) on 128-dim phase space.  Per batch row the heavy work is
    msel' = x @ W2 + d20          (W2 = S[rows, :64].T, shape (64, 20))
    out_k = log1p(msel'_x[k]^2 + msel'_p[k]^2 + cov_k/4 - 0.5)
i.e. a (B,64) @ (64,20) matmul + elementwise tail -> (B,10).  Memory bound:
minimize HBM bytes (fp16 in, fp16 out; gate is 2e-2, fp16 end-to-end is
~5e-4) and keep the 16 DMA engines saturated end-to-end.

Device layout (per core, R = 125184 rows = 489 pair-blocks of 256):
  - host packs xstack (128, R/2) fp16, "2-pack": column c = (pair b,
    lane l), partitions 0..63 = features of row 256b+l, partitions
    64..127 = features of row 256b+128+l.  Full 128 partitions keeps
    DMA descriptors on all 16 engines (a 65-partition layout was
    measured to use only 13 and run ~25% slower per descriptor) and
    halves the LDWEIGHTS count (one stationary load per 256 rows).
  - per super-block of `jblk` pair-blocks: 1 DMA [128, 128*jblk] fp16
    (12 KB/partition descriptors at jblk=48 — the shape measured at
    ~19.7 B/ns/engine), one matmul per pair-block: stationary =
    xstack_b [128, 128], moving = wcat [128, 40] = [[W2,0],[0,W2]],
    psum cols = [Ax Ap Bx Bp] x 10.  12 pair-blocks per 512-col psum
    bank (480 cols used), up to 4 banks/super-block, double-buffered.
  - tail: t2 = psum + d (DVE, fp16 out), sq = t2^2 (ACT), s = sq_x +
    sq_p (DVE fp16), v = s + covc (DVE fp16), o = ln(1+v) (ACT, fp16).
    relu is dropped: nmean >= 0 exactly (mean photon number), and v is
    a sum of nonnegative fp16 terms so ln(1+v) is always finite.
  - out DMA [128, 20*jblk] fp16 on the scalar HWDGE queue: output never
    queues behind input loads and there is no gpsimd SWDGE drain.
  - widths taper [12, 24, 48*8, 36, 24, 9] pair-blocks: small first
    block starts compute early; small last blocks shrink the post-DMA
    pipeline drain.
"""

import ml_dtypes
import numpy as np

import concourse.bacc as bacc
import concourse.mybir as mybir
import concourse.tile as tile
from concourse.bass_utils import run_bass_kernel_spmd

N = 64          # wires
OUT = 10        # measured wires / classes
NCORES = 8
PPB = 12                       # pair-blocks per psum bank (12*40 = 480 cols)
WIDTHS = [12, 24] + [48] * 8 + [36, 18, 9, 6]  # pair-blocks per super-block
NP2 = sum(WIDTHS)              # 489 pair-blocks
NJ = 2 * NP2                   # 978 j-blocks of 128 rows
R = 128 * NJ                   # per-core rows = 125184
B_PAD = R * NCORES             # 1001472
F32 = mybir.dt.float32
F16 = mybir.dt.float16
BF16 = mybir.dt.bfloat16
NPF16 = np.float16
NPBF16 = ml_dtypes.bfloat16


# ---------------------------------------------------------------- host math
def _bs_pass(n, start, int_params):
    i = np.arange(start, n - 1, 2)
    j = i + 1
    theta = int_params[3 * i]
    phi = int_params[3 * i + 1]
    ct, st = np.cos(theta), np.sin(theta)
    cp, sp = np.cos(phi), np.sin(phi)
    S = np.eye(2 * n)
    S[i, i] = ct
    S[i, j] = -cp * st
    S[i, n + j] = -sp * st
    S[j, i] = cp * st
    S[j, j] = ct
    S[j, n + i] = -sp * st
    S[n + i, j] = sp * st
    S[n + i, n + i] = ct
    S[n + i, n + j] = -cp * st
    S[n + j, i] = sp * st
    S[n + j, n + i] = cp * st
    S[n + j, n + j] = ct
    return S


def _layer_symplectic(n, int1, squeezes, int2):
    M = _bs_pass(n, 0, int1)
    M = _bs_pass(n, 1, int1) @ M
    c = np.concatenate([np.cos(int1[2::3]), np.ones(1)])
    s = np.concatenate([np.sin(int1[2::3]), np.zeros(1)])
    Rm = np.block([[np.diag(c), np.diag(-s)], [np.diag(s), np.diag(c)]])
    Sq = np.diag(np.concatenate([np.exp(-squeezes), np.exp(squeezes)]))
    M = Sq @ (Rm @ M)
    M = _bs_pass(n, 0, int2) @ M
    M = _bs_pass(n, 1, int2) @ M
    return M


def _affine_map(layers):
    n = N
    S = np.eye(2 * n)
    d = np.zeros(2 * n)
    for int1, sq, int2, disp in layers:
        M = _layer_symplectic(n, int1, sq, int2)
        S = M @ S
        d = M @ d
        d[:n] += 2.0 * disp
    return S, d


def _device_constants(layers):
    S, d = _affine_map(layers)
    w = np.arange(OUT)
    rows = np.concatenate([w, N + w])
    cov = S @ S.T
    cov_term = cov[w, w] + cov[N + w, N + w]            # (10,)
    W2 = S[rows, :N].T                                  # (64, 20), msel' scale
    d20 = d[rows] / 2.0                                 # (20,)
    covc = (cov_term / 4.0 - 0.5).astype(np.float32)    # (10,)

    wcat = np.zeros((128, 40), NPBF16)                   # [[W2, 0], [0, W2]]
    wcat[0:64, 0:20] = W2.astype(NPBF16)
    wcat[64:128, 20:40] = W2.astype(NPBF16)

    dconst = np.ascontiguousarray(np.broadcast_to(
        np.tile(d20.astype(np.float32), 2 * 4 * PPB),
        (128, 40 * 4 * PPB))).astype(np.float32)
    cconst = np.ascontiguousarray(np.broadcast_to(
        np.tile(covc, 2 * 4 * PPB), (128, 20 * 4 * PPB))).astype(NPF16)
    return wcat, dconst, cconst


# ---------------------------------------------------------------- bass build
def build_nc(widths=None):
    widths = widths or WIDTHS
    np2 = sum(widths)
    cc = 128 * np2                              # xstack cols
    nc = bacc.Bacc("TRN2", target_bir_lowering=False)
    WC = 40 * 4 * PPB                           # psum cols per full SB (1920)
    OC = 20 * 4 * PPB                           # out cols per full SB (960)
    xs = nc.dram_tensor("xs", (128, cc), BF16, kind="ExternalInput")
    wst = nc.dram_tensor("wcat", (128, 40), BF16, kind="ExternalInput")
    dcon = nc.dram_tensor("dconst", (128, WC), F32, kind="ExternalInput")
    ccon = nc.dram_tensor("covconst", (128, OC), F16, kind="ExternalInput")
    out = nc.dram_tensor("out", (128, 2 * np2 * OUT), F16,
                         kind="ExternalOutput")

    Square = mybir.ActivationFunctionType.Square
    Ln = mybir.ActivationFunctionType.Ln

    with tile.TileContext(nc) as tc:
        with (
            tc.tile_pool(name="const", bufs=1) as cpool,
            tc.tile_pool(name="xin", bufs=8) as xpool,
            tc.tile_pool(name="mid", bufs=3) as mpool,
            tc.tile_pool(name="ob", bufs=4) as opool,
            tc.tile_pool(name="ps", bufs=4, space="PSUM") as pspool,
        ):
            # w_t gates the first matmul: load it first on the sync queue;
            # d/c consts go on the scalar queue so they don't delay x.
            w_t = cpool.tile([128, 40], BF16)
            nc.sync.dma_start(w_t[:], wst[:])
            d_t = cpool.tile([128, WC], F32)
            nc.scalar.dma_start(d_t[:], dcon[:])
            c_t = cpool.tile([128, OC], F16)
            nc.scalar.dma_start(c_t[:], ccon[:])

            def emit_tail(prev):
                # tail of the PREVIOUS super-block: by now its sq is long
                # done, so none of these ops ever stalls at the head of an
                # in-order engine queue (the queues run strictly in order,
                # so an op waiting on a slow cross-engine dependency blocks
                # everything emitted after it on the same queue).
                sq, col_base, jblk = prev
                oc = 20 * jblk
                sqv = sq[:].rearrange("p (g r k) -> p g r k", r=2, k=OUT)
                s = mpool.tile([128, oc], F16, tag="s")
                sv = s[:].rearrange("p (g k) -> p g k", k=OUT)
                nc.vector.tensor_add(sv, sqv[:, :, 0, :], sqv[:, :, 1, :])
                v = mpool.tile([128, oc], F16, tag="v")
                nc.vector.tensor_add(v[:], s[:], c_t[:, 0:oc])
                o = opool.tile([128, oc], F16, tag="o")
                nc.scalar.activation(o[:], v[:], Ln, bias=1.0)
                ob = (col_base // 128) * 20
                nc.scalar.dma_start(out[:, ob:ob + oc], o[:])

            def emit_sb(col_base, jblk, in_chunks, prev):
                wc = 40 * jblk
                w = 128 * jblk
                tin = xpool.tile([128, w], BF16, tag="tin")
                q = w // in_chunks
                for c4 in range(in_chunks):
                    nc.sync.dma_start(
                        tin[:, c4 * q:(c4 + 1) * q],
                        xs[:, col_base + c4 * q:col_base + (c4 + 1) * q])

                if prev is not None:
                    emit_tail(prev)

                # psum: 12 pair-blocks use the first 480 cols of each
                # 512-col bank (no bank crossing).  Chunks of <= 24 pairs
                # (2 banks) with bufs=4 keep two super-blocks of matmuls
                # in flight and recycle banks as soon as the per-chunk
                # d-add has drained them.
                t2 = mpool.tile([128, wc], F16, tag="t2")
                base = 0
                left = jblk
                while left:
                    cp = min(2 * PPB, left)
                    nb = (cp + PPB - 1) // PPB
                    ps = pspool.tile([128, nb, 512], F32, tag="ps")
                    for j in range(cp):
                        jj = base + j
                        nc.tensor.matmul(
                            ps[:, j // PPB, 40 * (j % PPB):40 * (j % PPB) + 40],
                            tin[:, 128 * jj:128 * jj + 128], w_t[:],
                            start=True, stop=True,
                        )
                    fullb = cp // PPB
                    remj = cp - fullb * PPB
                    off = 40 * base
                    if fullb:
                        pv = ps[:, 0:fullb, 0:40 * PPB]
                        tv = t2[:, off:off + 40 * fullb * PPB].rearrange(
                            "p (t q) -> p t q", t=fullb)
                        dv = d_t[:, 0:40 * fullb * PPB].rearrange(
                            "p (t q) -> p t q", t=fullb)
                        nc.vector.tensor_add(tv, pv, dv)
                    if remj:
                        pv = ps[:, fullb, 0:40 * remj]
                        tv = t2[:, off + 40 * fullb * PPB:off + 40 * cp]
                        nc.vector.tensor_add(tv, pv, d_t[:, 0:40 * remj])
                    base += cp
                    left -= cp

                sq = mpool.tile([128, wc], F16, tag="sq")
                nc.scalar.activation(sq[:], t2[:], Square)
                return (sq, col_base, jblk)

            # first tile's DMA in halves so compute starts sooner
            col = 0
            prev = None
            for i, wdt in enumerate(widths):
                prev = emit_sb(col, wdt, 2 if i == 0 else 1, prev)
                col += 128 * wdt
            emit_tail(prev)
    nc.compile()
    return nc


# ---------------------------------------------------------------- host glue
def _make_in_maps(x_batch, wcat, dconst, cconst):
    B = x_batch.shape[0]
    xpad = np.zeros((B_PAD, N), NPBF16)
    xpad[:B] = x_batch
    in_maps = []
    for c in range(NCORES):
        xc = xpad[c * R:(c + 1) * R]
        # xstk[64*m + f, 128*b + l] = xc[256*b + 128*m + l, f]
        xstk = np.ascontiguousarray(
            xc.reshape(R // 256, 2, 128, N).transpose(1, 3, 0, 2)
            .reshape(128, R // 2))
        in_maps.append({"xs": xstk, "wcat": wcat,
                        "dconst": dconst, "covconst": cconst})
    return in_maps


def _decode_out(results, B):
    full = np.empty((B_PAD, OUT), np.float32)
    for c in range(NCORES):
        O = results[c]["out"].astype(np.float32).reshape(128, NJ, OUT)
        rows = O.transpose(1, 0, 2).reshape(R, OUT)
        full[c * R:(c + 1) * R] = rows
    return full[:B]


_NC_CACHE = {}


def kernel(x_batch, int1_0, squeezes_0, int2_0, disp_0,
           int1_1, squeezes_1, int2_1, disp_1, _trace=False):
    layers = [
        (np.asarray(int1_0, np.float64), np.asarray(squeezes_0, np.float64),
         np.asarray(int2_0, np.float64), np.asarray(disp_0, np.float64)),
        (np.asarray(int1_1, np.float64), np.asarray(squeezes_1, np.float64),
         np.asarray(int2_1, np.float64), np.asarray(disp_1, np.float64)),
    ]
    wcat, dconst, cconst = _device_constants(layers)
    in_maps = _make_in_maps(np.asarray(x_batch, np.float32), wcat, dconst,
                            cconst)

    if "nc" not in _NC_CACHE:
        _NC_CACHE["nc"] = build_nc()
    nc = _NC_CACHE["nc"]

    res = run_bass_kernel_spmd(
        nc, in_maps, core_ids=list(range(NCORES)), trace=_trace
    )
    out = _decode_out(res.results, x_batch.shape[0])
    if _trace:
        return out, res
    return out


# revision 13
# speedup vs baseline: 1.1156x; 1.0471x over previous
"""CVQNN classifier kernel for 8 Trainium2 NeuronCores.

Math: the whole quantum circuit collapses to a batch-independent affine map
(S, d) on 128-dim phase space.  Per batch row the heavy work is
    msel' = x @ W2 + d20          (W2 = S[rows, :64].T, shape (64, 20))
    out_k = log1p(msel'_x[k]^2 + msel'_p[k]^2 + cov_k/4 - 0.5)
i.e. a (B,64) @ (64,20) matmul + elementwise tail -> (B,10).  Memory bound:
minimize HBM bytes (fp16 in, fp16 out; gate is 2e-2, fp16 end-to-end is
~5e-4) and keep the 16 DMA engines saturated end-to-end.

Device layout (per core, R = 125184 rows = 489 pair-blocks of 256):
  - host packs xstack (128, R/2) fp16, "2-pack": column c = (pair b,
    lane l), partitions 0..63 = features of row 256b+l, partitions
    64..127 = features of row 256b+128+l.  Full 128 partitions keeps
    DMA descriptors on all 16 engines (a 65-partition layout was
    measured to use only 13 and run ~25% slower per descriptor) and
    halves the LDWEIGHTS count (one stationary load per 256 rows).
  - per super-block of `jblk` pair-blocks: 1 DMA [128, 128*jblk] fp16
    (12 KB/partition descriptors at jblk=48 — the shape measured at
    ~19.7 B/ns/engine), one matmul per pair-block: stationary =
    xstack_b [128, 128], moving = wcat [128, 40] = [[W2,0],[0,W2]],
    psum cols = [Ax Ap Bx Bp] x 10.  12 pair-blocks per 512-col psum
    bank (480 cols used), up to 4 banks/super-block, double-buffered.
  - tail: t2 = psum + d (DVE, fp16 out), sq = t2^2 (ACT), s = sq_x +
    sq_p (DVE fp16), v = s + covc (DVE fp16), o = ln(1+v) (ACT, fp16).
    relu is dropped: nmean >= 0 exactly (mean photon number), and v is
    a sum of nonnegative fp16 terms so ln(1+v) is always finite.
  - out DMA [128, 20*jblk] fp16 on the scalar HWDGE queue: output never
    queues behind input loads and there is no gpsimd SWDGE drain.
  - widths taper [12, 24, 48*8, 36, 24, 9] pair-blocks: small first
    block starts compute early; small last blocks shrink the post-DMA
    pipeline drain.
"""

import ml_dtypes
import numpy as np

import concourse.bacc as bacc
import concourse.mybir as mybir
import concourse.tile as tile
from concourse.bass_utils import run_bass_kernel_spmd

N = 64          # wires
OUT = 10        # measured wires / classes
NCORES = 8
PPB = 12                       # pair-blocks per psum bank (12*40 = 480 cols)
WIDTHS = [12, 24] + [48] * 8 + [36, 18, 9, 6]  # pair-blocks per super-block
NP2 = sum(WIDTHS)              # 489 pair-blocks
NJ = 2 * NP2                   # 978 j-blocks of 128 rows
R = 128 * NJ                   # per-core rows = 125184
B_PAD = R * NCORES             # 1001472
F32 = mybir.dt.float32
F16 = mybir.dt.float16
BF16 = mybir.dt.bfloat16
NPF16 = np.float16
NPBF16 = ml_dtypes.bfloat16


# ---------------------------------------------------------------- host math
def _bs_pass(n, start, int_params):
    i = np.arange(start, n - 1, 2)
    j = i + 1
    theta = int_params[3 * i]
    phi = int_params[3 * i + 1]
    ct, st = np.cos(theta), np.sin(theta)
    cp, sp = np.cos(phi), np.sin(phi)
    S = np.eye(2 * n)
    S[i, i] = ct
    S[i, j] = -cp * st
    S[i, n + j] = -sp * st
    S[j, i] = cp * st
    S[j, j] = ct
    S[j, n + i] = -sp * st
    S[n + i, j] = sp * st
    S[n + i, n + i] = ct
    S[n + i, n + j] = -cp * st
    S[n + j, i] = sp * st
    S[n + j, n + i] = cp * st
    S[n + j, n + j] = ct
    return S


def _layer_symplectic(n, int1, squeezes, int2):
    M = _bs_pass(n, 0, int1)
    M = _bs_pass(n, 1, int1) @ M
    c = np.concatenate([np.cos(int1[2::3]), np.ones(1)])
    s = np.concatenate([np.sin(int1[2::3]), np.zeros(1)])
    Rm = np.block([[np.diag(c), np.diag(-s)], [np.diag(s), np.diag(c)]])
    Sq = np.diag(np.concatenate([np.exp(-squeezes), np.exp(squeezes)]))
    M = Sq @ (Rm @ M)
    M = _bs_pass(n, 0, int2) @ M
    M = _bs_pass(n, 1, int2) @ M
    return M


def _affine_map(layers):
    n = N
    S = np.eye(2 * n)
    d = np.zeros(2 * n)
    for int1, sq, int2, disp in layers:
        M = _layer_symplectic(n, int1, sq, int2)
        S = M @ S
        d = M @ d
        d[:n] += 2.0 * disp
    return S, d


def _device_constants(layers):
    S, d = _affine_map(layers)
    w = np.arange(OUT)
    rows = np.concatenate([w, N + w])
    cov = S @ S.T
    cov_term = cov[w, w] + cov[N + w, N + w]            # (10,)
    W2 = S[rows, :N].T                                  # (64, 20), msel' scale
    d20 = d[rows] / 2.0                                 # (20,)
    covc = (cov_term / 4.0 - 0.5).astype(np.float32)    # (10,)

    wcat = np.zeros((128, 40), NPBF16)                   # [[W2, 0], [0, W2]]
    wcat[0:64, 0:20] = W2.astype(NPBF16)
    wcat[64:128, 20:40] = W2.astype(NPBF16)

    dconst = np.ascontiguousarray(np.broadcast_to(
        np.tile(d20.astype(np.float32), 2 * 4 * PPB),
        (128, 40 * 4 * PPB))).astype(np.float32)
    cconst = np.ascontiguousarray(np.broadcast_to(
        np.tile(covc, 2 * 4 * PPB), (128, 20 * 4 * PPB))).astype(NPF16)
    return wcat, dconst, cconst


# ---------------------------------------------------------------- bass build
def build_nc(widths=None):
    widths = widths or WIDTHS
    np2 = sum(widths)
    cc = 128 * np2                              # xstack cols
    nc = bacc.Bacc("TRN2", target_bir_lowering=False)
    WC = 40 * 4 * PPB                           # psum cols per full SB (1920)
    OC = 20 * 4 * PPB                           # out cols per full SB (960)
    xs = nc.dram_tensor("xs", (128, cc), BF16, kind="ExternalInput")
    wst = nc.dram_tensor("wcat", (128, 40), BF16, kind="ExternalInput")
    dcon = nc.dram_tensor("dconst", (128, WC), F32, kind="ExternalInput")
    ccon = nc.dram_tensor("covconst", (128, OC), F16, kind="ExternalInput")
    out = nc.dram_tensor("out", (128, 2 * np2 * OUT), F16,
                         kind="ExternalOutput")

    Square = mybir.ActivationFunctionType.Square
    Ln = mybir.ActivationFunctionType.Ln

    with tile.TileContext(nc) as tc:
        with (
            tc.tile_pool(name="const", bufs=1) as cpool,
            tc.tile_pool(name="xin", bufs=8) as xpool,
            tc.tile_pool(name="mid", bufs=5) as mpool,
            tc.tile_pool(name="ob", bufs=6) as opool,
            tc.tile_pool(name="ps", bufs=4, space="PSUM") as pspool,
        ):
            # w_t gates the first matmul: load it first on the sync queue;
            # d/c consts go on the scalar queue so they don't delay x.
            w_t = cpool.tile([128, 40], BF16)
            nc.sync.dma_start(w_t[:], wst[:])
            d_t = cpool.tile([128, WC], F32)
            nc.scalar.dma_start(d_t[:], dcon[:])
            c_t = cpool.tile([128, OC], F16)
            nc.scalar.dma_start(c_t[:], ccon[:])

            def emit_tail(prev):
                # tail of the PREVIOUS super-block: by now its sq is long
                # done, so none of these ops ever stalls at the head of an
                # in-order engine queue (the queues run strictly in order,
                # so an op waiting on a slow cross-engine dependency blocks
                # everything emitted after it on the same queue).
                sq, col_base, jblk = prev
                oc = 20 * jblk
                sqv = sq[:].rearrange("p (g r k) -> p g r k", r=2, k=OUT)
                s = mpool.tile([128, oc], F16, tag="s")
                sv = s[:].rearrange("p (g k) -> p g k", k=OUT)
                nc.vector.tensor_add(sv, sqv[:, :, 0, :], sqv[:, :, 1, :])
                v = mpool.tile([128, oc], F16, tag="v")
                nc.vector.tensor_add(v[:], s[:], c_t[:, 0:oc])
                o = opool.tile([128, oc], F16, tag="o")
                nc.scalar.activation(o[:], v[:], Ln, bias=1.0)
                ob = (col_base // 128) * 20
                nc.gpsimd.dma_start(out[:, ob:ob + oc], o[:])

            def emit_sb(col_base, jblk, in_chunks, prev):
                wc = 40 * jblk
                w = 128 * jblk
                tin = xpool.tile([128, w], BF16, tag="tin")
                q = w // in_chunks
                for c4 in range(in_chunks):
                    nc.sync.dma_start(
                        tin[:, c4 * q:(c4 + 1) * q],
                        xs[:, col_base + c4 * q:col_base + (c4 + 1) * q])

                if prev is not None:
                    emit_tail(prev)

                # psum: 12 pair-blocks use the first 480 cols of each
                # 512-col bank (no bank crossing).  Chunks of <= 24 pairs
                # (2 banks) with bufs=4 keep two super-blocks of matmuls
                # in flight and recycle banks as soon as the per-chunk
                # d-add has drained them.
                t2 = mpool.tile([128, wc], F16, tag="t2")
                base = 0
                left = jblk
                while left:
                    cp = min(2 * PPB, left)
                    nb = (cp + PPB - 1) // PPB
                    ps = pspool.tile([128, nb, 512], F32, tag="ps")
                    for j in range(cp):
                        jj = base + j
                        nc.tensor.matmul(
                            ps[:, j // PPB, 40 * (j % PPB):40 * (j % PPB) + 40],
                            tin[:, 128 * jj:128 * jj + 128], w_t[:],
                            start=True, stop=True,
                        )
                    fullb = cp // PPB
                    remj = cp - fullb * PPB
                    off = 40 * base
                    if fullb:
                        pv = ps[:, 0:fullb, 0:40 * PPB]
                        tv = t2[:, off:off + 40 * fullb * PPB].rearrange(
                            "p (t q) -> p t q", t=fullb)
                        dv = d_t[:, 0:40 * fullb * PPB].rearrange(
                            "p (t q) -> p t q", t=fullb)
                        nc.vector.tensor_add(tv, pv, dv)
                    if remj:
                        pv = ps[:, fullb, 0:40 * remj]
                        tv = t2[:, off + 40 * fullb * PPB:off + 40 * cp]
                        nc.vector.tensor_add(tv, pv, d_t[:, 0:40 * remj])
                    base += cp
                    left -= cp

                sq = mpool.tile([128, wc], F16, tag="sq")
                nc.scalar.activation(sq[:], t2[:], Square)
                return (sq, col_base, jblk)

            # first tile's DMA in halves so compute starts sooner
            col = 0
            prev = None
            for i, wdt in enumerate(widths):
                prev = emit_sb(col, wdt, 2 if wdt >= 2 * PPB else 1, prev)
                col += 128 * wdt
            emit_tail(prev)
    nc.compile()
    return nc


# ---------------------------------------------------------------- host glue
def _make_in_maps(x_batch, wcat, dconst, cconst):
    B = x_batch.shape[0]
    xpad = np.zeros((B_PAD, N), NPBF16)
    xpad[:B] = x_batch
    in_maps = []
    for c in range(NCORES):
        xc = xpad[c * R:(c + 1) * R]
        # xstk[64*m + f, 128*b + l] = xc[256*b + 128*m + l, f]
        xstk = np.ascontiguousarray(
            xc.reshape(R // 256, 2, 128, N).transpose(1, 3, 0, 2)
            .reshape(128, R // 2))
        in_maps.append({"xs": xstk, "wcat": wcat,
                        "dconst": dconst, "covconst": cconst})
    return in_maps


def _decode_out(results, B):
    full = np.empty((B_PAD, OUT), np.float32)
    for c in range(NCORES):
        O = results[c]["out"].astype(np.float32).reshape(128, NJ, OUT)
        rows = O.transpose(1, 0, 2).reshape(R, OUT)
        full[c * R:(c + 1) * R] = rows
    return full[:B]


_NC_CACHE = {}


def kernel(x_batch, int1_0, squeezes_0, int2_0, disp_0,
           int1_1, squeezes_1, int2_1, disp_1, _trace=False):
    layers = [
        (np.asarray(int1_0, np.float64), np.asarray(squeezes_0, np.float64),
         np.asarray(int2_0, np.float64), np.asarray(disp_0, np.float64)),
        (np.asarray(int1_1, np.float64), np.asarray(squeezes_1, np.float64),
         np.asarray(int2_1, np.float64), np.asarray(disp_1, np.float64)),
    ]
    wcat, dconst, cconst = _device_constants(layers)
    in_maps = _make_in_maps(np.asarray(x_batch, np.float32), wcat, dconst,
                            cconst)

    if "nc" not in _NC_CACHE:
        _NC_CACHE["nc"] = build_nc()
    nc = _NC_CACHE["nc"]

    res = run_bass_kernel_spmd(
        nc, in_maps, core_ids=list(range(NCORES)), trace=_trace
    )
    out = _decode_out(res.results, x_batch.shape[0])
    if _trace:
        return out, res
    return out
